# revision 29
# baseline (speedup 1.0000x reference)
"""Trainium2 Bass kernel for 12-head attention (SEQ=4096, D=768), 8-core SPMD.

Sharding: head-parallel with a sequence-split remainder. Core c owns full head
A_c = c and the half of head B_c = 8 + c//2 selected by (c % 2). Upper-half
cores receive a sequence-rolled copy of x so every core's program computes
local queries [0, 2048) for its B head (pure SPMD, no divergent control flow).
Each core returns a partial output projection [768, 4096] (fp16); the host
un-rolls, transposes and sums the 8 partials.

Per-core dataflow (matmuls in fp16, softmax internals in fp32):
  x^T -> QKV^T projections -> scores S^T[j,i] = K^T(lhsT) x Q^T(rhs), the two
  heads of a pair computed concurrently on disjoint PE row groups
  -> exp on ScalarE (scale=1/8 folded; no max subtraction, scores in [-10,10])
  -> attn@V with [V | ones] stationary so softmax denominators fall out as an
  extra PSUM row -> normalize via K=2 broadcast matmul + fast reciprocal ->
  output projection.

Schedule: the ScalarE exp stream (192 x [128,1024] ACTIVATEs ~ 204 us) is the
critical resource. Everything else is arranged to hide under it:
  - x is DMA'd chunk-major and only chunks 0/1 are projected up front; the
    remaining 6 chunks' projections are injected into the pair-0 key-block
    loop so the first exp fires within ~10 us of kernel start.
  - the exp table is preloaded by a dummy activation at t~0.
  - phase 3 (denominator broadcast, normalize, output projection, store) is
    split into small steps injected into the pair-1/2 loops, reusing the
    scores PSUM rotation (tag "s"); only the last pair's 4 chunks drain at
    the end, with PSUM->SBUF copies alternating Vector/Scalar engines.
"""

import numpy as np

N_CORES = 8
N_HEADS = 12
HEAD_DIM = 64
N_FEATS = 768
SEQ = 4096
FCH = N_FEATS // 128  # contraction chunks of the feature dim
W = 1024              # i-chunk width (exp granularity)
NIC = SEQ // W
NJB = SEQ // 128      # key blocks
NH = W // 512         # 512-wide matmul sub-chunks per i-chunk
NCH = SEQ // 512

_PROGRAM = None
LAST_RESULT = None


def _build_program():
    import concourse.tile as tile
    from concourse import bacc, mybir

    f32 = mybir.dt.float32
    f32r = mybir.dt.float32r
    f16 = mybir.dt.float16
    EXP = mybir.ActivationFunctionType.Exp
    MULT = mybir.AluOpType.mult
    ADD = mybir.AluOpType.add

    nc = bacc.Bacc("TRN2", target_bir_lowering=False, debug=False,
                   num_devices=N_CORES)

    xt_d = nc.dram_tensor("xt", [N_FEATS, SEQ], f16, kind="ExternalInput").ap()
    wqk_d = nc.dram_tensor("wqk", [N_FEATS, 256], f16, kind="ExternalInput").ap()
    wv_d = nc.dram_tensor("wv", [N_FEATS, 128], f16, kind="ExternalInput").ap()
    wo_d = nc.dram_tensor("wo", [128, N_FEATS], f16, kind="ExternalInput").ap()
    sel_d = nc.dram_tensor("sel", [2, 128], f32r, kind="ExternalInput").ap()
    id_d = nc.dram_tensor("ident", [128, 128], f16, kind="ExternalInput").ap()
    out_d = nc.dram_tensor("out", [N_FEATS, SEQ], f16, kind="ExternalOutput").ap()

    with tile.TileContext(nc) as tc:
        with tc.tile_pool(name="persist", bufs=1) as pp, \
             tc.tile_pool(name="ps_s", space="PSUM", bufs=2) as ps_s, \
             tc.tile_pool(name="ps_ov", space="PSUM", bufs=1) as ps_ov, \
             tc.tile_pool(name="exps", bufs=12) as pe, \
             tc.tile_pool(name="ph3", bufs=2) as p3:

            # ---- persistent tiles ----
            wqk_sb = pp.tile([128, FCH, 256], f16)
            wv_sb = pp.tile([128, FCH, 128], f16)
            wo_sb = pp.tile([128, N_FEATS], f16)
            sel_sb = pp.tile([66, 128], f32r)
            id_sb = pp.tile([128, 128], f16)
            dummy = pp.tile([128, 16], f32)
            scratch = pp.tile([128, 128], f32)
            q_t = [pp.tile([128, 512], f16, name=f"q_t{i}") for i in range(NCH)]
            k_t = [pp.tile([128, 512], f16, name=f"k_t{i}") for i in range(NCH)]
            q2_t = [pp.tile([128, 512], f16, name=f"q2_t{i}") for i in range(NCH)]
            k2_t = [pp.tile([128, 512], f16, name=f"k2_t{i}") for i in range(NCH)]
            vA_t = pp.tile([128, NJB, 65], f16)
            vB_t = pp.tile([128, NJB, 66], f16)
            vT_t = [pp.tile([128, 512], f16, name=f"vT_t{i}") for i in range(NCH)]
            xt = pp.tile([128, FCH, SEQ], f16)
            attn_out = pp.tile([128, SEQ], f32)  # rows 0-63 A dims, 64-127 B dims
            den = pp.tile([66, SEQ], f32r)       # rows 64 (A), 65 (B)

            # exp table preload: first ACTIVATE triggers ACT_TABLE_LOAD early
            nc.vector.memset(dummy[:], 0.0)
            nc.scalar.activation(out=dummy[:, 8:16], in_=dummy[:, 0:8], func=EXP)

            # ---- input DMAs split across the two queues ----
            xt_r = xt_d.rearrange("(c p) n -> p c n", p=128)
            nc.sync.dma_start(out=wqk_sb[:], in_=wqk_d.rearrange("(c p) m -> p c m", p=128))
            nc.gpsimd.dma_start(out=id_sb[:], in_=id_d[:])
            nc.gpsimd.dma_start(out=wv_sb[:], in_=wv_d.rearrange("(c p) m -> p c m", p=128))
            nc.gpsimd.dma_start(out=wo_sb[:], in_=wo_d[:])
            nc.gpsimd.dma_start(out=sel_sb[64:66, :], in_=sel_d[:])
            for cp in range(SEQ // 1024):
                for k in range(FCH):
                    eng = nc.sync if k < 3 else nc.gpsimd
                    eng.dma_start(out=xt[:, k, cp * 1024:(cp + 1) * 1024],
                                  in_=xt_r[:, k, cp * 1024:(cp + 1) * 1024])

            # ---- projection streams ----
            def proj_qk(nch):
                pq = ps_s.tile([128, W], f32, tag="s", name=f"pjqk{nch}")
                for k in range(FCH):
                    nc.tensor.matmul(pq[:, 0:512], wqk_sb[:, k, 0:128],
                                     xt[:, k, nch * 512:(nch + 1) * 512],
                                     start=(k == 0), stop=(k == FCH - 1))
                    nc.tensor.matmul(pq[:, 512:1024], wqk_sb[:, k, 128:256],
                                     xt[:, k, nch * 512:(nch + 1) * 512],
                                     start=(k == 0), stop=(k == FCH - 1))
                nc.vector.tensor_copy(q_t[nch][:], pq[:, 0:512])
                nc.vector.tensor_copy(k_t[nch][:], pq[:, 512:1024])
                nc.gpsimd.dma_start(out=q2_t[nch][64:128, :], in_=q_t[nch][0:64, :])
                nc.gpsimd.dma_start(out=k2_t[nch][64:128, :], in_=k_t[nch][0:64, :])

            def _trans(nch):
                ptt = ps_s.tile([128, 4, 128], f16, tag="s", name=f"ptr{nch}")
                for q in range(4):
                    nc.tensor.transpose(ptt[:, q, :], vT_t[nch][:, q * 128:(q + 1) * 128], id_sb[:])
                jb0 = nch * 4
                nc.vector.tensor_copy(vA_t[:, jb0:jb0 + 4, 0:64], ptt[:, 0:4, 0:64])
                nc.vector.tensor_copy(vB_t[:, jb0:jb0 + 4, 0:64], ptt[:, 0:4, 64:128])

            def proj_v(nch):
                pv = ps_s.tile([128, W], f32, tag="s", name=f"pjv{nch}")
                for k in range(FCH):
                    nc.tensor.matmul(pv[:, 0:512], wv_sb[:, k, :],
                                     xt[:, k, nch * 512:(nch + 1) * 512],
                                     start=(k == 0), stop=(k == FCH - 1))
                nc.vector.tensor_copy(vT_t[nch][:], pv[:, 0:512])
                _trans(nch)

            # 3-matmul pieces: PSUM partial + vector-engine combine
            part_state = {}

            def _half_p1(c, key, wt, lo, hi):
                ps = ps_s.tile([128, 512], f32, tag="s", name=f"pp1{key}{c}")
                for k in range(3):
                    nc.tensor.matmul(ps[:], wt[:, k, lo:hi],
                                     xt[:, k, c * 512:(c + 1) * 512],
                                     start=(k == 0), stop=(k == 2))
                pt = p3.tile([128, 512], f32, tag=f"part_{key}", name=f"pt{key}{c}", bufs=2)
                nc.vector.tensor_copy(pt[:], ps[:])
                part_state[(key, c)] = pt

            def _half_p2(c, key, wt, lo, hi, dest):
                ps = ps_s.tile([128, 512], f32, tag="s", name=f"pp2{key}{c}")
                for k in range(3, FCH):
                    nc.tensor.matmul(ps[:], wt[:, k, lo:hi],
                                     xt[:, k, c * 512:(c + 1) * 512],
                                     start=(k == 3), stop=(k == FCH - 1))
                nc.vector.tensor_tensor(out=dest[:], in0=ps[:],
                                        in1=part_state.pop((key, c))[:], op=ADD)

            def piece_k1(c): _half_p1(c, "k", wqk_sb, 128, 256)

            def piece_k2(c):
                _half_p2(c, "k", wqk_sb, 128, 256, k_t[c])
                nc.gpsimd.dma_start(out=k2_t[c][64:128, :], in_=k_t[c][0:64, :])

            def piece_q1(c): _half_p1(c, "q", wqk_sb, 0, 128)

            def piece_q2(c):
                _half_p2(c, "q", wqk_sb, 0, 128, q_t[c])
                nc.gpsimd.dma_start(out=q2_t[c][64:128, :], in_=q_t[c][0:64, :])

            def piece_v1(c): _half_p1(c, "v", wv_sb, 0, 128)

            def piece_v2(c): _half_p2(c, "v", wv_sb, 0, 128, vT_t[c])

            def piece_tr(c): _trans(c)

            # ---- phase 3 on the freed attn@V accumulator banks ----
            # At a pair boundary, allocate one [128, W] tile per chunk on the
            # ov tags (slot 0 = den broadcast, slot 1 = out-proj ping) and emit
            # the broadcast/reciprocal/normalize immediately; the 6 output-
            # projection matmuls per chunk are emitted later, spread over the
            # next pair's key blocks, so they never head-block the tensor FIFO
            # and never touch the scores rotation.
            def ph3_alloc(ts):
                tiles = []
                for i, t in enumerate(ts):
                    g = ps_ov.tile([128, W], f32, tag=("ov1" if i % 2 == 0 else "ov2"),
                                   name=f"bcpo{t}")
                    tiles.append((t, g))
                return tiles

            def ph3_emit(tiles):
                steps = []
                for t, g in tiles:
                    t0 = t * 512
                    nc.tensor.matmul(g[:, 0:512], sel_sb[64:66, :],
                                     den[64:66, t0:t0 + 512], start=True, stop=True)
                    rc = p3.tile([128, 512], f32, tag="rc", name=f"rc{t}", bufs=3)
                    nc.vector.reciprocal_approx_fast(out=rc[:], in_=g[:, 0:512])
                    nm = p3.tile([128, 512], f16, tag="nm", name=f"nm{t}", bufs=5)
                    nc.vector.tensor_tensor(out=nm[:], in0=attn_out[:, t0:t0 + 512],
                                            in1=rc[:], op=MULT)
                    for fb in range(6):
                        steps.append((t, g, nm, fb))
                # round-robin chunks so every tag's tile is released early
                order = []
                for fb in range(6):
                    for i in range(len(tiles)):
                        order.append(steps[i * 6 + fb])
                return order

            def ph3_step(st, tail=False):
                t, g, nm, fb = st
                t0 = t * 512
                nc.tensor.matmul(g[:, 512:1024], wo_sb[:, fb * 128:(fb + 1) * 128],
                                 nm[:], start=True, stop=True)
                ob = p3.tile([128, 512], f16, tag="ob", name=f"ob{t}_{fb}", bufs=6)
                if tail and fb % 2 == 1:
                    nc.scalar.copy(ob[:], g[:, 512:1024])
                else:
                    nc.vector.tensor_copy(ob[:], g[:, 512:1024])
                nc.sync.dma_start(out=out_d[fb * 128:(fb + 1) * 128, t0:t0 + 512],
                                  in_=ob[:])

            # ---- PE warm-up (HAM) during the x DMA, then pair-0 q/k and the
            # first key block's scores/exp (starts the exp stream early) ----
            wm = ps_s.tile([128, 128], f32, tag="s", name="warm")
            for i in range(20):
                nc.tensor.matmul(wm[:], id_sb[:], id_sb[:], start=(i == 0), stop=(i == 19))
            nc.vector.tensor_copy(scratch[:], wm[:])

            pairs = [
                (("A", 0, q_t, k_t, vA_t, 65), ("B", 0, q_t, k_t, vB_t, 66)),
                (("A", 2, q_t, k_t, vA_t, 65), ("A2", 3, q2_t, k2_t, vA_t, 65)),
                (("A", 1, q_t, k_t, vA_t, 65), ("B", 1, q_t, k_t, vB_t, 66)),
            ]

            pre_e = {}

            def _score_e(pi, jb, h, c1, c2):
                jc, jo = jb // 4, (jb % 4) * 128
                sp = ps_s.tile([128, W], f32, tag="s", name=f"s{pi}_{jb}_{h}")
                for ci, (_, ic, qt, kt, _, _) in enumerate((c1, c2)):
                    base = ci * 64
                    nc.tensor.matmul(
                        sp[:, ci * 512:(ci + 1) * 512],
                        kt[jc][base:base + 64, jo:jo + 128],
                        qt[ic * NH + h][base:base + 64, :],
                        start=True, stop=True)
                e = pe.tile([128, W], f16, tag="e", name=f"e{pi}_{jb}_{h}")
                nc.scalar.activation(out=e[:], in_=sp[:], func=EXP, scale=0.125)
                return e

            proj_qk(0)
            pre_e[(0, 0, 0)] = _score_e(0, 0, 0, *pairs[0])
            proj_qk(1)
            pre_e[(0, 0, 1)] = _score_e(0, 0, 1, *pairs[0])
            # bulk constants after the critical q/k drains
            nc.vector.memset(vA_t[:, :, 64:65], 1.0)
            nc.vector.memset(vB_t[:, :, 64:65], 0.0)
            nc.vector.memset(vB_t[:, :, 65:66], 1.0)
            nc.vector.memset(den[64:66, SEQ // 2:].bitcast(f32), 1.0)
            nc.vector.memset(attn_out[64:128, SEQ // 2:], 0.0)
            pre_e[(0, 1, 0)] = _score_e(0, 1, 0, *pairs[0])
            pre_e[(0, 1, 1)] = _score_e(0, 1, 1, *pairs[0])

            # first-pair injection timetable: paired pieces keep the scores
            # PSUM rotation parity and stay under the per-block tensor slack
            inj0 = {
                0: [(piece_v1, 0), (piece_v2, 0), (piece_tr, 0)],
                1: [(piece_v1, 1), (piece_v2, 1)],
                2: [(piece_tr, 1), (piece_k1, 2)],
                3: [(piece_k2, 2), (piece_v1, 2)],
                4: [(piece_v2, 2), (piece_tr, 2)],
                5: [(piece_k1, 3), (piece_k2, 3)],
                6: [(piece_v1, 3), (piece_v2, 3)],
                7: [(piece_tr, 3), (piece_k1, 4)],
                8: [(piece_k2, 4), (piece_v1, 4)],
                9: [(piece_v2, 4), (piece_tr, 4)],
                10: [(piece_k1, 5), (piece_k2, 5)],
                11: [(piece_v1, 5), (piece_v2, 5)],
                12: [(piece_tr, 5), (piece_k1, 6)],
                13: [(piece_k2, 6), (piece_v1, 6)],
                14: [(piece_v2, 6), (piece_tr, 6)],
                15: [(piece_k1, 7), (piece_k2, 7)],
                16: [(piece_v1, 7), (piece_v2, 7)],
                17: [(piece_tr, 7)],
                24: [(piece_q1, 6), (piece_q2, 6)],
                25: [(piece_q1, 7), (piece_q2, 7)],
                26: [(piece_q1, 4), (piece_q2, 4)],
                27: [(piece_q1, 5), (piece_q2, 5)],
            }
            # second pair: q for chunks 2,3 (needed by the third pair)
            inj1 = {
                13: [(piece_q1, 2), (piece_q2, 2)],
                15: [(piece_q1, 3), (piece_q2, 3)],
            }

            ph3_steps = []
            ph3_pending = []
            for pi, (c1, c2) in enumerate(pairs):
                ov1 = ps_ov.tile([128, W], f32, tag="ov1", name=f"ov1_{pi}")
                ov2 = ps_ov.tile([128, W], f32, tag="ov2", name=f"ov2_{pi}")
                ovs = (ov1, ov2)
                for jb in range(NJB):
                    if pi == 0:
                        for fn, c in inj0.get(jb, ()):
                            fn(c)
                    else:
                        if pi == 1:
                            for fn, c in inj1.get(jb, ()):
                                fn(c)
                        if jb > 0:
                            for _ in range(3):
                                if ph3_steps:
                                    ph3_step(ph3_steps.pop(0))
                    etiles = []
                    for h in range(NH):
                        e = pre_e.pop((pi, jb, h), None)
                        if e is None:
                            e = _score_e(pi, jb, h, c1, c2)
                        etiles.append(e)
                    if pi >= 1 and jb == 0 and ph3_pending:
                        ph3_steps = ph3_emit(ph3_pending)
                        ph3_pending = []
                    for h, e in enumerate(etiles):
                        for ci, (_, ic, _, _, vt, m) in enumerate((c1, c2)):
                            nc.tensor.matmul(ovs[ci][0:m, h * 512:(h + 1) * 512],
                                             vt[:, jb, 0:m], e[:, ci * 512:(ci + 1) * 512],
                                             start=(jb == 0), stop=(jb == NJB - 1))
                # pre-emit the next pair's first scores/exp so the exp
                # stream runs through the boundary drain window
                if pi + 1 < len(pairs):
                    for h in range(NH):
                        pre_e[(pi + 1, 0, h)] = _score_e(pi + 1, 0, h, *pairs[pi + 1])
                # drain accumulators to SBUF
                (n1, ic1, _, _, _, _), (n2, ic2, _, _, _, _) = c1, c2
                p10, p20 = ic1 * W, ic2 * W
                if n2 == "B":
                    nc.vector.tensor_copy(den[64:66, p20:p20 + W], ov2[64:66, :])
                    nc.vector.tensor_copy(den[64:65, p10:p10 + W], ov1[64:65, :])
                    nc.vector.tensor_copy(attn_out[0:64, p10:p10 + W], ov1[0:64, :])
                    ovb_sb = pe.tile([64, W], f32, tag="ovb_sb", name=f"ovb_sb{pi}", bufs=2)
                    nc.vector.tensor_copy(ovb_sb[:], ov2[0:64, :])
                    nc.gpsimd.dma_start(out=attn_out[64:128, p20:p20 + W], in_=ovb_sb[:])
                else:
                    nc.vector.tensor_copy(den[64:65, p10:p10 + W], ov1[64:65, :])
                    nc.vector.tensor_copy(den[64:65, p20:p20 + W], ov2[64:65, :])
                    nc.vector.tensor_copy(attn_out[0:64, p10:p10 + W], ov1[0:64, :])
                    nc.vector.tensor_copy(attn_out[0:64, p20:p20 + W], ov2[0:64, :])
                # open this pair's phase-3 group; its out-proj matmuls run
                # spread over the next pair's key blocks
                if pi == 0:
                    ph3_pending = ph3_alloc([0, 1])
                elif pi == 1:
                    ph3_pending = ph3_alloc([4, 5])

            # ---- tail: remaining phase-3 chunks, pipelined on freed banks ----
            for st in ph3_steps:
                ph3_step(st, tail=True)
            tails = (6, 7, 2, 3)
            nms = {}
            for i, t in enumerate(tails):
                t0 = t * 512
                bcg = ps_s.tile([128, 512], f32, tag="s", name=f"tbc{t}")
                nc.tensor.matmul(bcg[:], sel_sb[64:66, :], den[64:66, t0:t0 + 512],
                                 start=True, stop=True)
                rc = p3.tile([128, 512], f32, tag="rc", name=f"trc{t}", bufs=3)
                nc.vector.reciprocal_approx_fast(out=rc[:], in_=bcg[:])
                nm = p3.tile([128, 512], f16, tag="nm", name=f"tnm{t}", bufs=5)
                nc.vector.tensor_tensor(out=nm[:], in0=attn_out[:, t0:t0 + 512],
                                        in1=rc[:], op=MULT)
                nms[t] = nm
            tcnt = 0
            for fb in range(6):
                for t in tails:
                    t0 = t * 512
                    ptag = ("s", "ov2", "ov1")[tcnt % 3]
                    pool = ps_s if ptag == "s" else ps_ov
                    po = pool.tile([128, 512], f32, tag=ptag, name=f"tpo{t}_{fb}")
                    nc.tensor.matmul(po[:], wo_sb[:, fb * 128:(fb + 1) * 128],
                                     nms[t][:], start=True, stop=True)
                    ob = p3.tile([128, 512], f16, tag="ob", name=f"tob{t}_{fb}", bufs=6)
                    if tcnt % 3 == 1:
                        nc.scalar.copy(ob[:], po[:])
                    else:
                        nc.vector.tensor_copy(ob[:], po[:])
                    nc.sync.dma_start(out=out_d[fb * 128:(fb + 1) * 128, t0:t0 + 512],
                                      in_=ob[:])
                    tcnt += 1

    nc.compile()
    return nc


def _get_program():
    global _PROGRAM
    if _PROGRAM is None:
        _PROGRAM = _build_program()
    return _PROGRAM


def kernel(x: np.ndarray, w_qkv: np.ndarray, w_out: np.ndarray) -> np.ndarray:
    global LAST_RESULT
    import os
    try:
        import antenv.axon_hooks  # noqa: F401
    except ImportError:
        # without the NTFF hook, a leaked BASS_TRACE=1 would crash the
        # axon trace path inside run_bass_kernel_spmd
        os.environ["BASS_NEVER_TRACE"] = "1"
    from concourse.bass_utils import run_bass_kernel_spmd

    nc = _get_program()
    x2 = np.ascontiguousarray(x[0], dtype=np.float32)          # [SEQ, F]
    w_qkv = np.asarray(w_qkv, dtype=np.float32)                # [2304, F]
    w_out = np.asarray(w_out, dtype=np.float32)                # [F, 768]

    # per-head slices of w_qkv rows: o = h*192 + d*3 + {0:q, 1:k, 2:v}
    def wslice(h, which):
        return w_qkv[h * 192 + which:(h + 1) * 192:3, :]       # [64, F]

    ident = np.eye(128, dtype=np.float16)
    sel = np.zeros((2, 128), dtype=np.float32)
    sel[0, 0:64] = 1.0
    sel[1, 64:128] = 1.0

    xt_plain = np.ascontiguousarray(x2.T.astype(np.float16))   # [F, SEQ]
    xt_rolled = np.ascontiguousarray(np.roll(x2, -SEQ // 2, axis=0).T.astype(np.float16))

    in_maps = []
    rolls = []
    for c in range(N_CORES):
        hA = c
        hB = 8 + c // 2
        roll = (SEQ // 2) if (c % 2) else 0
        rolls.append(roll)
        wqk = np.ascontiguousarray(np.concatenate(
            [wslice(hA, 0), wslice(hB, 0), wslice(hA, 1), wslice(hB, 1)],
            axis=0).T.astype(np.float16))
        wv = np.ascontiguousarray(np.concatenate(
            [wslice(hA, 2), wslice(hB, 2)], axis=0).T.astype(np.float16))
        cols = list(range(hA * 64, hA * 64 + 64)) + list(range(hB * 64, hB * 64 + 64))
        wo = np.ascontiguousarray(w_out[:, cols].T.astype(np.float16))  # [128, F]
        in_maps.append({
            "xt": xt_rolled if roll else xt_plain,
            "wqk": wqk, "wv": wv, "wo": wo, "sel": sel, "ident": ident,
        })

    res = run_bass_kernel_spmd(nc, in_maps, list(range(N_CORES)))
    LAST_RESULT = res

    acc = np.zeros((SEQ, N_FEATS), dtype=np.float64)
    for c in range(N_CORES):
        part = res.results[c]["out"]                           # [F, SEQ] fp16
        if rolls[c]:
            part = np.roll(part, rolls[c], axis=1)
        acc += part.T.astype(np.float64)
    return acc.astype(np.float32)[None]
